# revision 32
# baseline (speedup 1.0000x reference)
"""Diag-embed kernel for Trainium2 (raw Bass, manual semaphores).

Problem: x [8192, 176] f32 -> out [8192, 176, 176] f32 with
out[i] = diag(x[i]).  Data-parallel over 8 NeuronCores: core c handles
batch rows [1024c, 1024(c+1)).

Primary scheme ("amx" + single_packet, see PRIMARY below): the runtime
pre-zeroes ExternalOutput DRAM, so only the 1024*176 diagonal f32 per
core are written, each as one 32 B-aligned full-word DMA descriptor
(diag j sits at float j%8 of its 8-float output group; the staged SBUF
copy surrounds it with zeros so the 32 B write needs no HBM
read-modify-write).  With item b staged on SBUF partition b//8 slot
b%8, each of the 8 phase classes (j%8) of the whole shard is ONE
[1024 item, 22 window, 8 float] store DMA - 4 DMAs per HWDGE ring,
issued with single_packet=True (packs each DMA's descriptor ring
stream into larger packets, ~0.5%).  The SP phases 0-3 / ACT 4-7
split makes concurrently-drained descriptors 2816 B = 11x256 B apart:
an integer channel stride coprime with the 16 HBM channels, i.e.
perfect channel round-robin (even/odd or chunk splits measure 4-8%
worse).  kernel() verifies the zero-fill contract on host (exact diag
+ nonzero count) and falls back to the dense 127 MB writer if it
doesn't hold.

Measured: all 32 B scatter layouts sit on the SDMA per-descriptor floor
(~9.7 ns/desc/engine; 180224 descs over 16 engines/core ~= 109 us);
dense writes are HBM-bandwidth-bound (~353 us) and 4 B descriptors pay
~4x HBM read-modify-write (~428 us).

The older dense/segment, chunked-scatter and hybrid builders below are
kept for A/B benchmarking and as the dense fallback.
"""

import numpy as np

B_FULL = 8192
D = 176
DD = D * D            # 30976 floats per item
N_CORES = 8
B_SHARD = B_FULL // N_CORES   # 1024
P = 128
N_CHUNKS = B_SHARD // P       # 8

SEGMENTS = 8          # DD % SEGMENTS == 0; templates total 121 KiB/partition

_prog_cache = {}


def _segment_diag(s: int, W: int):
    """(j0, cnt, c0): diag indices [j0, j0+cnt) fall in columns
    [s*W, (s+1)*W) of the flat item row, at in-segment offset
    c0 + k*(D+1)."""
    j0 = -(-(s * W) // (D + 1))                 # ceil
    j1 = ((s + 1) * W - 1) // (D + 1)           # floor, inclusive
    return j0, j1 - j0 + 1, j0 * (D + 1) - s * W


def _build_program(repeat: int = 1, timing: bool = False, segments: int = SEGMENTS):
    """repeat>1 re-runs the whole store pipeline (same output region)
    inside one NEFF.  timing=True redirects the big output to an internal
    DRAM scratch tensor (same HBM-write work) and exposes only a tiny
    [128,1] ExternalOutput, so benchmarking doesn't ship 1 GB over the
    axon relay.  Both knobs are for test.py only."""
    from concourse import bass, mybir

    f32 = mybir.dt.float32
    S = segments
    assert DD % S == 0
    W = DD // S
    nc = bass.Bass(target_bir_lowering=False)

    x = nc.dram_tensor("x", [B_SHARD, D], f32, kind="ExternalInput")
    if timing:
        out = nc.dram_tensor("outscratch", [B_SHARD, D, D], f32)
        tiny = nc.dram_tensor("tiny_out", [P, 1], f32, kind="ExternalOutput")
    else:
        out = nc.dram_tensor("out", [B_SHARD, D, D], f32, kind="ExternalOutput")
        tiny = None
    out2d = out[:].rearrange("b i j -> b (i j)")   # [1024, 30976]

    import contextlib

    with contextlib.ExitStack() as ctx:
        sem_x = ctx.enter_context(nc.semaphore("sem_x"))
        sem_t = ctx.enter_context(nc.semaphore("sem_t"))
        sem_d = [ctx.enter_context(nc.semaphore(f"sem_d{s}")) for s in range(S)]
        sem_s = [ctx.enter_context(nc.semaphore(f"sem_s{s}")) for s in range(S)]
        tmpl = [
            ctx.enter_context(nc.sbuf_tensor(f"t{s}", [P, W], f32))
            for s in range(S)
        ]
        xall = ctx.enter_context(
            nc.sbuf_tensor("xall", [P, N_CHUNKS, D], f32)
        )
        diag = [_segment_diag(s, W) for s in range(S)]

        # GpSimd is unused; skip its expensive dge_drain in the end barrier
        with nc.Block(no_gpsimd_drain=True) as block:

            # stores ride BOTH HWDGE rings (SP and ACT): when one ring's head
            # waits on a scatter sem the other keeps the SDMA engines fed
            def store_stream(eng, segs):
                for m in range(N_CHUNKS * repeat):
                    n = m % N_CHUNKS
                    rows = slice(n * P, (n + 1) * P)
                    for s in segs:
                        dma = eng.dma_start(
                            out=out2d[rows, s * W : (s + 1) * W], in_=tmpl[s][:]
                        )
                        dma.wait_op(sem_s[s], m + 1, "sem-ge")   # RAW: scatter
                        dma.then_inc(sem_d[s], 16)
                # all stores landed before the end-of-kernel barrier
                for s in segs:
                    eng.wait_ge(sem_d[s], 16 * N_CHUNKS * repeat)

            @block.scalar
            def _(act):
                # per-chunk x loads on the ACT HWDGE queue (parallel to the
                # store queue); chunk 0's 90 KB load unblocks the pipeline
                for n in range(N_CHUNKS):
                    act.dma_start(
                        out=xall[:, n, :], in_=x[n * P : (n + 1) * P, :]
                    ).then_inc(sem_x, 16)
                store_stream(act, range(S // 2, S))

            @block.vector
            def _(v):
                for m in range(N_CHUNKS * repeat):
                    n = m % N_CHUNKS
                    for s in range(S):
                        j0, cnt, c0 = diag[s]
                        if m == 0:
                            # interleave zero-fills with the first chunk's
                            # scatters so dma_s(0) can start right after
                            # memset s instead of after all S memsets
                            v.memset(tmpl[s][:], 0.0)
                        i = v.tensor_copy(
                            tmpl[s][:, c0 : c0 + (cnt - 1) * (D + 1) + 1 : D + 1],
                            xall[:, n, j0 : j0 + cnt],
                        )
                        if m == 0:
                            if s == 0:
                                i.wait_op(sem_x, 16, "sem-ge")   # chunk 0's x
                            elif s == S - 1:
                                # guard: every later scatter follows this one
                                # in DVE program order, so all x is resident
                                i.wait_op(sem_x, 16 * N_CHUNKS, "sem-ge")
                        else:
                            i.wait_op(sem_d[s], 16 * m, "sem-ge")  # WAR
                        i.then_inc(sem_s[s])

            @block.sync
            def _(sp):
                store_stream(sp, range(S // 2))
                if tiny is not None:
                    dt_ = sp.dma_start(out=tiny[:], in_=tmpl[0][:, 0:1])
                    dt_.then_inc(sem_t, 16)
                    sp.wait_ge(sem_t, 16)

    return nc


def _get_program(repeat: int = 1, timing: bool = False, segments: int = SEGMENTS):
    key = ("nc", repeat, timing, segments)
    if key not in _prog_cache:
        _prog_cache[key] = _build_program(repeat, timing, segments)
    return _prog_cache[key]


def _build_scatter_program(repeat: int = 1, timing: bool = False):
    """Diag-scatter-only kernel: relies on the runtime contract that
    ExternalOutput DRAM is zero-filled before the NEFF runs (bass2jax
    binds np.zeros to the output tensor as an input; the native
    run_bass_kernel_spmd path pre-zeros ExternalOutput buffers —
    "kernels that don't write every element rely on that").  So only the
    1024*176 diagonal f32s per core are written, as one strided-dest DMA
    per 128-row chunk: dst elements 4 B each at stride 177 floats.
    kernel() verifies the zero contract on host and falls back to the
    dense program if it doesn't hold.
    """
    from concourse import bass, mybir

    f32 = mybir.dt.float32
    nc = bass.Bass(target_bir_lowering=False)

    x = nc.dram_tensor("x", [B_SHARD, D], f32, kind="ExternalInput")
    if timing:
        out = nc.dram_tensor("outscratch", [B_SHARD, D, D], f32)
        tiny = nc.dram_tensor("tiny_out", [P, 1], f32, kind="ExternalOutput")
    else:
        out = nc.dram_tensor("out", [B_SHARD, D, D], f32, kind="ExternalOutput")
        tiny = None
    out2d = out[:].rearrange("b i j -> b (i j)")   # [1024, 30976]
    dstep = D + 1
    dlast = (D - 1) * dstep + 1                    # 30976: 176 diag slots

    import contextlib

    with contextlib.ExitStack() as ctx:
        sem_x = ctx.enter_context(nc.semaphore("sem_x"))
        sem_t = ctx.enter_context(nc.semaphore("sem_t"))
        sem_dsp = ctx.enter_context(nc.semaphore("sem_dsp"))
        sem_dact = ctx.enter_context(nc.semaphore("sem_dact"))
        xall = ctx.enter_context(
            nc.sbuf_tensor("xall", [P, N_CHUNKS, D], f32)
        )

        with nc.Block(no_gpsimd_drain=True) as block:

            def store_stream(eng, chunks, sem_d):
                cnt = 0
                for _ in range(repeat):
                    for n in chunks:
                        rows = slice(n * P, (n + 1) * P)
                        with nc.allow_non_contiguous_dma(
                            reason="diag scatter: 4B elements at stride 177"
                        ):
                            dma = eng.dma_start(
                                out=out2d[rows, 0:dlast:dstep],
                                in_=xall[:, n, :],
                            )
                        if cnt < len(chunks):
                            # first pass: wait for this chunk's x load
                            dma.wait_op(sem_x, 16 * (n + 1), "sem-ge")
                        dma.then_inc(sem_d, 16)
                        cnt += 1
                eng.wait_ge(sem_d, 16 * cnt)

            @block.scalar
            def _(act):
                for n in range(N_CHUNKS):
                    act.dma_start(
                        out=xall[:, n, :], in_=x[n * P : (n + 1) * P, :]
                    ).then_inc(sem_x, 16)
                store_stream(act, range(N_CHUNKS // 2, N_CHUNKS), sem_dact)

            @block.sync
            def _(sp):
                store_stream(sp, range(N_CHUNKS // 2), sem_dsp)
                if tiny is not None:
                    dt_ = sp.dma_start(out=tiny[:], in_=xall[:, 0, 0:1])
                    dt_.wait_op(sem_x, 16, "sem-ge")
                    dt_.then_inc(sem_t, 16)
                    sp.wait_ge(sem_t, 16)

    return nc


def _get_scatter_program(repeat: int = 1, timing: bool = False):
    key = ("sc", repeat, timing)
    if key not in _prog_cache:
        _prog_cache[key] = _build_scatter_program(repeat, timing)
    return _prog_cache[key]


def _build_hybrid_program(
    repeat: int = 1,
    timing: bool = False,
    a: int = 88,
    s_dense: int = 4,
    aligned: bool = False,
):
    """Hybrid diag writer over pre-zeroed output.

    Diags [0, a) are DMA-scattered (tiny descriptors; cost ~ descriptor
    count).  Diags [a, 176) are covered by a dense template band (cost ~
    HBM write bytes), split into s_dense segments pipelined across the
    two HWDGE rings exactly like the dense kernel.  a trades descriptor
    work against byte work.

    aligned=True stages the scattered diags into 32 B windows
    (x_j at float j%8, rest zeros of the output row) so every scatter
    descriptor is one aligned 32 B full-word write: same descriptor
    count, but no HBM read-modify-write.  Requires a % 8 == 0.
    """
    from concourse import bass, mybir

    f32 = mybir.dt.float32
    nc = bass.Bass(target_bir_lowering=False)
    assert 0 <= a <= D
    if aligned:
        assert a % 8 == 0
    if a < D:
        assert s_dense >= 1 and (D - a) % s_dense == 0
        g = (D - a) // s_dense
    else:
        s_dense, g = 0, 0

    x = nc.dram_tensor("x", [B_SHARD, D], f32, kind="ExternalInput")
    if timing:
        out = nc.dram_tensor("outscratch", [B_SHARD, D, D], f32)
        tiny = nc.dram_tensor("tiny_out", [P, 1], f32, kind="ExternalOutput")
    else:
        out = nc.dram_tensor("out", [B_SHARD, D, D], f32, kind="ExternalOutput")
        tiny = None
    out2d = out[:].rearrange("b i j -> b (i j)")   # [1024, 30976]
    dstep = D + 1

    # dense segments: seg s covers flat cols [col0, col0+width), holding
    # g diag slots at local offsets t*dstep
    segs = []
    for s in range(s_dense):
        j0 = a + s * g
        col0 = dstep * j0
        col1 = dstep * (j0 + g) if s < s_dense - 1 else DD
        segs.append((col0, col1 - col0))

    import contextlib

    with contextlib.ExitStack() as ctx:
        sem_x = ctx.enter_context(nc.semaphore("sem_x"))
        sem_t = ctx.enter_context(nc.semaphore("sem_t"))
        sem_d = [ctx.enter_context(nc.semaphore(f"sem_d{s}")) for s in range(s_dense)]
        sem_s = [ctx.enter_context(nc.semaphore(f"sem_s{s}")) for s in range(s_dense)]
        sem_scsp = ctx.enter_context(nc.semaphore("sem_scsp"))
        sem_scact = ctx.enter_context(nc.semaphore("sem_scact"))
        sem_xs = ctx.enter_context(nc.semaphore("sem_xs")) if aligned else None
        xall = ctx.enter_context(nc.sbuf_tensor("xall", [P, N_CHUNKS, D], f32))
        tmpl = [
            ctx.enter_context(nc.sbuf_tensor(f"t{s}", [P, w], f32))
            for s, (_, w) in enumerate(segs)
        ]
        xs = (
            ctx.enter_context(nc.sbuf_tensor("xs", [P, N_CHUNKS, a * 8], f32))
            if aligned and a
            else None
        )

        with nc.Block(no_gpsimd_drain=True) as block:

            def scatter_dma(eng, n, jlo, jhi):
                """one scatter store for diags [jlo, jhi) of chunk n"""
                rows = slice(n * P, (n + 1) * P)
                if not aligned:
                    with nc.allow_non_contiguous_dma(
                        reason="diag scatter: 4B elements at stride 177"
                    ):
                        return eng.dma_start(
                            out=out2d[rows, dstep * jlo : dstep * (jhi - 1) + 1 : dstep],
                            in_=xall[:, n, jlo:jhi],
                        )
                # aligned: phases p = j%8, j = 8t+p in [jlo, jhi).  The
                # 32 B dst window for diag j starts at byte 5664*t+704*p
                # = 8-float group 177*t + 22*p of the item row (30976 =
                # 3872 groups of 8); x_j sits at in-window offset j%8.
                assert jlo % 8 == 0 and jhi % 8 == 0
                t0, t1 = jlo // 8, jhi // 8
                out3 = out2d[rows, :].rearrange("b (t f) -> b t f", f=8)
                xs3 = xs[:, n, :].rearrange("b (j f) -> b j f", f=8)
                dmas = []
                for p in range(8):
                    dst = out3[
                        :, 177 * t0 + 22 * p : 177 * (t1 - 1) + 22 * p + 1 : 177, :
                    ]
                    src = xs3[:, 8 * t0 + p : 8 * (t1 - 1) + p + 1 : 8, :]
                    dmas.append(eng.dma_start(out=dst, in_=src))
                return dmas

            def dense_store(eng, m, s):
                n = m % N_CHUNKS
                rows = slice(n * P, (n + 1) * P)
                col0, w = segs[s]
                return eng.dma_start(
                    out=out2d[rows, col0 : col0 + w], in_=tmpl[s][:]
                )

            n_sc_sp = 0
            n_sc_act = 0

            @block.scalar
            def _(act):
                nonlocal n_sc_act
                for n in range(N_CHUNKS):
                    act.dma_start(
                        out=xall[:, n, :], in_=x[n * P : (n + 1) * P, :]
                    ).then_inc(sem_x, 16)
                for m in range(N_CHUNKS * repeat):
                    n = m % N_CHUNKS
                    # scatter upper half of [0, a)
                    if a:
                        dmas = scatter_dma(act, n, a // 2, a)
                        dmas = dmas if isinstance(dmas, list) else [dmas]
                        for d_ in dmas:
                            if m < N_CHUNKS:
                                if aligned:
                                    d_.wait_op(sem_xs, 8 * (n + 1), "sem-ge")
                                else:
                                    d_.wait_op(sem_x, 16 * (n + 1), "sem-ge")
                            d_.then_inc(sem_scact, 16)
                            n_sc_act += 1
                    for s in range(s_dense // 2, s_dense):
                        dma = dense_store(act, m, s)
                        dma.wait_op(sem_s[s], m + 1, "sem-ge")
                        dma.then_inc(sem_d[s], 16)
                if a:
                    act.wait_ge(sem_scact, 16 * n_sc_act)
                for s in range(s_dense // 2, s_dense):
                    act.wait_ge(sem_d[s], 16 * N_CHUNKS * repeat)

            if s_dense or aligned:

                @block.vector
                def _(v):
                    if aligned:
                        v.memset(xs[:], 0.0)
                        for n in range(N_CHUNKS):
                            # stage x_j into float slot 8j + j%8, per
                            # phase p: slots 64t + 9p, t in [0, 22)
                            for p in range(8):
                                i = v.tensor_copy(
                                    xs[:, n, 9 * p : 9 * p + 64 * (a // 8 - 1) + 1 : 64],
                                    xall[:, n, p : p + 8 * (a // 8 - 1) + 1 : 8],
                                )
                                if p == 0:
                                    i.wait_op(sem_x, 16 * (n + 1), "sem-ge")
                                i.then_inc(sem_xs, 1)
                        # sem_xs counts 8 per chunk
                    for m in range(N_CHUNKS * repeat):
                        n = m % N_CHUNKS
                        for s in range(s_dense):
                            col0, w = segs[s]
                            if m == 0:
                                v.memset(tmpl[s][:], 0.0)
                            i = v.tensor_copy(
                                tmpl[s][:, 0 : dstep * (g - 1) + 1 : dstep],
                                xall[:, n, a + s * g : a + (s + 1) * g],
                            )
                            if m == 0:
                                if s == 0 and not aligned:
                                    i.wait_op(sem_x, 16, "sem-ge")
                                elif s == s_dense - 1 and not aligned:
                                    i.wait_op(sem_x, 16 * N_CHUNKS, "sem-ge")
                            else:
                                i.wait_op(sem_d[s], 16 * m, "sem-ge")
                            i.then_inc(sem_s[s])

            @block.sync
            def _(sp):
                nonlocal n_sc_sp
                for m in range(N_CHUNKS * repeat):
                    n = m % N_CHUNKS
                    if a:
                        dmas = scatter_dma(sp, n, 0, a // 2)
                        dmas = dmas if isinstance(dmas, list) else [dmas]
                        for d_ in dmas:
                            if m < N_CHUNKS:
                                if aligned:
                                    d_.wait_op(sem_xs, 8 * (n + 1), "sem-ge")
                                else:
                                    d_.wait_op(sem_x, 16 * (n + 1), "sem-ge")
                            d_.then_inc(sem_scsp, 16)
                            n_sc_sp += 1
                    for s in range(s_dense // 2):
                        dma = dense_store(sp, m, s)
                        dma.wait_op(sem_s[s], m + 1, "sem-ge")
                        dma.then_inc(sem_d[s], 16)
                if a:
                    sp.wait_ge(sem_scsp, 16 * n_sc_sp)
                for s in range(s_dense // 2):
                    sp.wait_ge(sem_d[s], 16 * N_CHUNKS * repeat)
                if tiny is not None:
                    dt_ = sp.dma_start(out=tiny[:], in_=xall[:, 0, 0:1])
                    dt_.then_inc(sem_t, 16)
                    sp.wait_ge(sem_t, 16)

    return nc


def _get_hybrid_program(repeat=1, timing=False, a=88, s_dense=4, aligned=False):
    key = ("hy", repeat, timing, a, s_dense, aligned)
    if key not in _prog_cache:
        _prog_cache[key] = _build_hybrid_program(repeat, timing, a, s_dense, aligned)
    return _prog_cache[key]


def _build_aligned_merged(repeat: int = 1, timing: bool = False):
    """Aligned diag scatter with one DMA per phase per iteration.

    Same 32 B-aligned windows as the aligned hybrid (diag j staged at
    in-window float j%8, window = 8-float group 177*t + 22*p of the item
    row, j = 8t+p), but the (chunk, t) lattice is expressed as one 4D AP
    [128 part, 8 chunks, 22 t, 8 f], so each iteration is just 8 DMA
    instructions (phases 0-3 on the SP ring, 4-7 on ACT) instead of 128.
    Relies on pre-zeroed ExternalOutput DRAM (see _build_scatter_program).
    """
    from concourse import bass, mybir

    f32 = mybir.dt.float32
    nc = bass.Bass(target_bir_lowering=False)

    x = nc.dram_tensor("x", [B_SHARD, D], f32, kind="ExternalInput")
    if timing:
        out = nc.dram_tensor("outscratch", [B_SHARD, D, D], f32)
        tiny = nc.dram_tensor("tiny_out", [P, 1], f32, kind="ExternalOutput")
    else:
        out = nc.dram_tensor("out", [B_SHARD, D, D], f32, kind="ExternalOutput")
        tiny = None
    out2d = out[:].rearrange("b i j -> b (i j)")   # [1024, 30976]
    T_PER_PHASE = D // 8            # 22 windows per phase per item

    import contextlib

    with contextlib.ExitStack() as ctx:
        sem_x = ctx.enter_context(nc.semaphore("sem_x"))
        sem_t = ctx.enter_context(nc.semaphore("sem_t"))
        sem_xs = ctx.enter_context(nc.semaphore("sem_xs"))
        sem_dsp = ctx.enter_context(nc.semaphore("sem_dsp"))
        sem_dact = ctx.enter_context(nc.semaphore("sem_dact"))
        xall = ctx.enter_context(nc.sbuf_tensor("xall", [P, N_CHUNKS, D], f32))
        xs = ctx.enter_context(nc.sbuf_tensor("xs", [P, N_CHUNKS, D * 8], f32))
        xs4 = xs[:].rearrange("b n (j f) -> b n j f", f=8)

        with nc.Block(no_gpsimd_drain=True) as block:

            def phase_store(eng, n, p):
                # 3D APs (DMA limit): [128 part, 22 t, 8 f]
                out3 = out2d[slice(n * P, (n + 1) * P), :].rearrange(
                    "b (t f) -> b t f", f=8
                )
                dst = out3[
                    :, 22 * p : 177 * (T_PER_PHASE - 1) + 22 * p + 1 : 177, :
                ]
                src = xs4[:, n, p : 8 * (T_PER_PHASE - 1) + p + 1 : 8, :]
                return eng.dma_start(out=dst, in_=src)

            def store_stream(eng, phases, sem_d):
                cnt = 0
                for r in range(repeat):
                    for n in range(N_CHUNKS):
                        for k, p in enumerate(phases):
                            dma = phase_store(eng, n, p)
                            if r == 0 and k == 0:
                                dma.wait_op(sem_xs, 8 * (n + 1), "sem-ge")
                            dma.then_inc(sem_d, 16)
                            cnt += 1
                eng.wait_ge(sem_d, 16 * cnt)

            @block.scalar
            def _(act):
                for n in range(N_CHUNKS):
                    act.dma_start(
                        out=xall[:, n, :], in_=x[n * P : (n + 1) * P, :]
                    ).then_inc(sem_x, 16)
                store_stream(act, [4, 5, 6, 7], sem_dact)

            @block.vector
            def _(v):
                v.memset(xs[:], 0.0)
                for n in range(N_CHUNKS):
                    # stage x_j -> xs flat slot 8j + j%8; per phase p the
                    # slots are 64t + 9p, t in [0, 22)
                    for p in range(8):
                        i = v.tensor_copy(
                            xs[:, n, 9 * p : 9 * p + 64 * (T_PER_PHASE - 1) + 1 : 64],
                            xall[:, n, p : p + 8 * (T_PER_PHASE - 1) + 1 : 8],
                        )
                        if p == 0:
                            i.wait_op(sem_x, 16 * (n + 1), "sem-ge")
                        i.then_inc(sem_xs, 1)

            @block.sync
            def _(sp):
                store_stream(sp, [0, 1, 2, 3], sem_dsp)
                if tiny is not None:
                    dt_ = sp.dma_start(out=tiny[:], in_=xall[:, 0, 0:1])
                    dt_.then_inc(sem_t, 16)
                    sp.wait_ge(sem_t, 16)

    return nc


def _get_aligned_merged(repeat: int = 1, timing: bool = False):
    key = ("am", repeat, timing)
    if key not in _prog_cache:
        _prog_cache[key] = _build_aligned_merged(repeat, timing)
    return _prog_cache[key]


def _build_aligned_streams(repeat: int = 1, timing: bool = False, mode: str = "3s"):
    """Aligned diag scatter with alternative work-to-stream layouts.

    mode="3s": three descriptor streams — SP gets diags [0,80), ACT
      [80,160), gpsimd/SWDGE [160,176) — tests whether HWDGE descriptor
      generation is the bottleneck (a third generator would relieve it).
    mode="cs": two streams, split by CHUNK (SP chunks 0-3, ACT 4-7, each
      full diag range, same [128,11,8] DMA shape as the best variant) —
      tests HBM bank spread (rings write different 15.9 MB regions).
    Same 32 B-aligned windows and staging as _build_hybrid_program.
    """
    from concourse import bass, mybir

    f32 = mybir.dt.float32
    nc = bass.Bass(target_bir_lowering=False)

    x = nc.dram_tensor("x", [B_SHARD, D], f32, kind="ExternalInput")
    if timing:
        out = nc.dram_tensor("outscratch", [B_SHARD, D, D], f32)
        tiny = nc.dram_tensor("tiny_out", [P, 1], f32, kind="ExternalOutput")
    else:
        out = nc.dram_tensor("out", [B_SHARD, D, D], f32, kind="ExternalOutput")
        tiny = None
    out2d = out[:].rearrange("b i j -> b (i j)")

    # per-stream issue plans: list of (n, p, t0, t1)
    if mode == "3s":
        ranges = {"sp": (0, 10), "act": (10, 20), "gp": (20, 22)}
        plans = {
            k: [(n, p, *ranges[k]) for n in range(N_CHUNKS) for p in range(8)]
            for k in ranges
        }
    elif mode == "cs":
        plans = {
            "sp": [
                (n, p, t0, t0 + 11)
                for n in range(N_CHUNKS // 2)
                for p in range(8)
                for t0 in (0, 11)
            ],
            "act": [
                (n, p, t0, t0 + 11)
                for n in range(N_CHUNKS // 2, N_CHUNKS)
                for p in range(8)
                for t0 in (0, 11)
            ],
            "gp": [],
        }
    elif mode in ("q", "e"):
        # same ring j-halves as the sc_al winner, but each phase DMA cut
        # into finer t-subranges: "q" ~quarters, "e" ~eighths
        subs = {
            "q": {"sp": [(0, 6), (6, 11)], "act": [(11, 17), (17, 22)]},
            "e": {
                "sp": [(0, 3), (3, 6), (6, 9), (9, 11)],
                "act": [(11, 14), (14, 17), (17, 20), (20, 22)],
            },
        }[mode]
        plans = {
            k: [
                (n, p, t0, t1)
                for n in range(N_CHUNKS)
                for p in range(8)
                for (t0, t1) in subs[k]
            ]
            for k in ("sp", "act")
        }
        plans["gp"] = []
    else:
        raise ValueError(mode)

    import contextlib

    with contextlib.ExitStack() as ctx:
        sem_x = ctx.enter_context(nc.semaphore("sem_x"))
        sem_t = ctx.enter_context(nc.semaphore("sem_t"))
        sem_xs = ctx.enter_context(nc.semaphore("sem_xs"))
        sem_d = {
            k: ctx.enter_context(nc.semaphore(f"sem_d_{k}")) for k in plans
        }
        xall = ctx.enter_context(nc.sbuf_tensor("xall", [P, N_CHUNKS, D], f32))
        xs = ctx.enter_context(nc.sbuf_tensor("xs", [P, N_CHUNKS, D * 8], f32))
        xs4 = xs[:].rearrange("b n (j f) -> b n j f", f=8)

        with nc.Block(no_gpsimd_drain=True) as block:

            def stream(eng, plan, sem):
                cnt = 0
                for r in range(repeat):
                    seen = set()
                    for (n, p, t0, t1) in plan:
                        out3 = out2d[slice(n * P, (n + 1) * P), :].rearrange(
                            "b (t f) -> b t f", f=8
                        )
                        dst = out3[
                            :, 22 * p + 177 * t0 : 177 * (t1 - 1) + 22 * p + 1 : 177, :
                        ]
                        src = xs4[:, n, 8 * t0 + p : 8 * (t1 - 1) + p + 1 : 8, :]
                        dma = eng.dma_start(out=dst, in_=src)
                        if r == 0 and n not in seen:
                            seen.add(n)
                            dma.wait_op(sem_xs, 8 * (n + 1), "sem-ge")
                        dma.then_inc(sem, 16)
                        cnt += 1
                if cnt:
                    eng.wait_ge(sem, 16 * cnt)

            @block.scalar
            def _(act):
                for n in range(N_CHUNKS):
                    act.dma_start(
                        out=xall[:, n, :], in_=x[n * P : (n + 1) * P, :]
                    ).then_inc(sem_x, 16)
                stream(act, plans["act"], sem_d["act"])

            if plans.get("gp"):

                @block.gpsimd
                def _(gp):
                    stream(gp, plans["gp"], sem_d["gp"])

            @block.vector
            def _(v):
                v.memset(xs[:], 0.0)
                for n in range(N_CHUNKS):
                    for p in range(8):
                        i = v.tensor_copy(
                            xs[:, n, 9 * p : 9 * p + 64 * 21 + 1 : 64],
                            xall[:, n, p : p + 8 * 21 + 1 : 8],
                        )
                        if p == 0:
                            i.wait_op(sem_x, 16 * (n + 1), "sem-ge")
                        i.then_inc(sem_xs, 1)

            @block.sync
            def _(sp):
                stream(sp, plans["sp"], sem_d["sp"])
                if tiny is not None:
                    dt_ = sp.dma_start(out=tiny[:], in_=xall[:, 0, 0:1])
                    dt_.then_inc(sem_t, 16)
                    sp.wait_ge(sem_t, 16)

    return nc


def _get_aligned_streams(repeat=1, timing=False, mode="3s"):
    key = ("as", repeat, timing, mode)
    if key not in _prog_cache:
        _prog_cache[key] = _build_aligned_streams(repeat, timing, mode)
    return _prog_cache[key]


def _build_am_split(repeat: int = 1, timing: bool = False,
                    sp=(0, 1, 2, 3), act=(4, 5, 6, 7), gp=()):
    """Aligned merged diag scatter with a configurable phase->stream map.

    Same windows/staging as _build_aligned_merged (one [128 part, 22 t,
    8 f] DMA per (chunk, phase)), but phases can be assigned to the SP
    HWDGE ring, the ACT HWDGE ring, or the gpsimd SWDGE queue (a third
    independent descriptor generator).
    """
    from concourse import bass, mybir

    f32 = mybir.dt.float32
    nc = bass.Bass(target_bir_lowering=False)
    assert sorted([*sp, *act, *gp]) == list(range(8))

    x = nc.dram_tensor("x", [B_SHARD, D], f32, kind="ExternalInput")
    if timing:
        out = nc.dram_tensor("outscratch", [B_SHARD, D, D], f32)
        tiny = nc.dram_tensor("tiny_out", [P, 1], f32, kind="ExternalOutput")
    else:
        out = nc.dram_tensor("out", [B_SHARD, D, D], f32, kind="ExternalOutput")
        tiny = None
    out2d = out[:].rearrange("b i j -> b (i j)")
    T_PER_PHASE = D // 8            # 22 windows per phase per item

    import contextlib

    with contextlib.ExitStack() as ctx:
        sem_x = ctx.enter_context(nc.semaphore("sem_x"))
        sem_t = ctx.enter_context(nc.semaphore("sem_t"))
        sem_xs = ctx.enter_context(nc.semaphore("sem_xs"))
        sem_dsp = ctx.enter_context(nc.semaphore("sem_dsp"))
        sem_dact = ctx.enter_context(nc.semaphore("sem_dact"))
        sem_dgp = ctx.enter_context(nc.semaphore("sem_dgp"))
        xall = ctx.enter_context(nc.sbuf_tensor("xall", [P, N_CHUNKS, D], f32))
        xs = ctx.enter_context(nc.sbuf_tensor("xs", [P, N_CHUNKS, D * 8], f32))
        xs4 = xs[:].rearrange("b n (j f) -> b n j f", f=8)

        with nc.Block(no_gpsimd_drain=not gp) as block:

            def phase_store(eng, n, p):
                out3 = out2d[slice(n * P, (n + 1) * P), :].rearrange(
                    "b (t f) -> b t f", f=8
                )
                dst = out3[
                    :, 22 * p : 177 * (T_PER_PHASE - 1) + 22 * p + 1 : 177, :
                ]
                src = xs4[:, n, p : 8 * (T_PER_PHASE - 1) + p + 1 : 8, :]
                return eng.dma_start(out=dst, in_=src)

            def store_stream(eng, phases, sem_d):
                cnt = 0
                for r in range(repeat):
                    for n in range(N_CHUNKS):
                        for k, p in enumerate(phases):
                            dma = phase_store(eng, n, p)
                            if r == 0 and k == 0:
                                dma.wait_op(sem_xs, 8 * (n + 1), "sem-ge")
                            dma.then_inc(sem_d, 16)
                            cnt += 1
                if cnt:
                    eng.wait_ge(sem_d, 16 * cnt)

            @block.scalar
            def _(act_e):
                for n in range(N_CHUNKS):
                    act_e.dma_start(
                        out=xall[:, n, :], in_=x[n * P : (n + 1) * P, :]
                    ).then_inc(sem_x, 16)
                store_stream(act_e, act, sem_dact)

            if gp:

                @block.gpsimd
                def _(g):
                    store_stream(g, gp, sem_dgp)

            @block.vector
            def _(v):
                v.memset(xs[:], 0.0)
                for n in range(N_CHUNKS):
                    for p in range(8):
                        i = v.tensor_copy(
                            xs[:, n, 9 * p : 9 * p + 64 * (T_PER_PHASE - 1) + 1 : 64],
                            xall[:, n, p : p + 8 * (T_PER_PHASE - 1) + 1 : 8],
                        )
                        if p == 0:
                            i.wait_op(sem_x, 16 * (n + 1), "sem-ge")
                        i.then_inc(sem_xs, 1)

            @block.sync
            def _(sp_e):
                store_stream(sp_e, sp, sem_dsp)
                if tiny is not None:
                    dt_ = sp_e.dma_start(out=tiny[:], in_=xall[:, 0, 0:1])
                    dt_.then_inc(sem_t, 16)
                    sp_e.wait_ge(sem_t, 16)

    return nc


def _get_am_split(repeat=1, timing=False, sp=(0, 1, 2, 3), act=(4, 5, 6, 7), gp=()):
    key = ("ams", repeat, timing, tuple(sp), tuple(act), tuple(gp))
    if key not in _prog_cache:
        _prog_cache[key] = _build_am_split(repeat, timing, sp, act, gp)
    return _prog_cache[key]


def _build_amx(repeat: int = 1, timing: bool = False, single_packet: bool = False,
               sp_ph=(0, 1, 2, 3), act_ph=(4, 5, 6, 7)):
    """Aligned diag scatter, transposed item->partition mapping: ONE
    store DMA per phase for the whole 1024-item shard.

    Item b is staged on SBUF partition b//8, slot b%8 (so the flat
    slot order (n, t, f) matches DRAM item order b = 8p+n).  Per phase
    ph the dst AP is [1024 items, 22 windows (stride 177 groups), 8 f]
    (3 dims) and the src is the contiguous [128, 1408] slice xs[:, ph]
    — so the whole shard is 8 store DMAs (4 per HWDGE ring) of 22528
    32 B-aligned descriptors each, vs 64 DMAs for the chunked layout.
    The x load is a single contiguous [128, 1408] DMA.  Relies on
    pre-zeroed ExternalOutput DRAM like the other scatter variants.
    """
    from concourse import bass, mybir

    f32 = mybir.dt.float32
    nc = bass.Bass(target_bir_lowering=False)
    NS = N_CHUNKS                  # 8 item slots per partition
    T = D // 8                     # 22 windows per phase per item

    x = nc.dram_tensor("x", [B_SHARD, D], f32, kind="ExternalInput")
    if timing:
        out = nc.dram_tensor("outscratch", [B_SHARD, D, D], f32)
        tiny = nc.dram_tensor("tiny_out", [P, 1], f32, kind="ExternalOutput")
    else:
        out = nc.dram_tensor("out", [B_SHARD, D, D], f32, kind="ExternalOutput")
        tiny = None
    # [1024 items, 3872 groups, 8 floats]
    outw = out[:].rearrange("b i j -> b (i j)").rearrange(
        "b (t f) -> b t f", f=8
    )

    import contextlib

    with contextlib.ExitStack() as ctx:
        sem_x = ctx.enter_context(nc.semaphore("sem_x"))
        sem_t = ctx.enter_context(nc.semaphore("sem_t"))
        sem_xs = ctx.enter_context(nc.semaphore("sem_xs"))
        sem_dsp = ctx.enter_context(nc.semaphore("sem_dsp"))
        sem_dact = ctx.enter_context(nc.semaphore("sem_dact"))
        xall = ctx.enter_context(nc.sbuf_tensor("xall", [P, NS, D], f32))
        # (p, phase, n, t, f): xs[:, ph] flat per partition = (n, t, f)
        # ascending = DRAM item order b = 8p + n with window t ascending
        xs = ctx.enter_context(nc.sbuf_tensor("xs", [P, 8, NS, T, 8], f32))

        with nc.Block(no_gpsimd_drain=True) as block:

            def phase_store(eng, ph):
                dst = outw[:, 22 * ph : 177 * (T - 1) + 22 * ph + 1 : 177, :]
                return eng.dma_start(
                    out=dst, in_=xs[:, ph], single_packet=single_packet
                )

            def store_stream(eng, phases, sem_d):
                cnt = 0
                for r in range(repeat):
                    for k, ph in enumerate(phases):
                        dma = phase_store(eng, ph)
                        if r == 0 and k == 0:
                            dma.wait_op(sem_xs, 8, "sem-ge")
                        dma.then_inc(sem_d, 16)
                        cnt += 1
                eng.wait_ge(sem_d, 16 * cnt)

            @block.scalar
            def _(act_e):
                # one contiguous load: partition p <- items [8p, 8p+8)
                act_e.dma_start(
                    out=xall[:],
                    in_=x[:].rearrange("(p n) d -> p (n d)", p=P),
                ).then_inc(sem_x, 16)
                if act_ph:
                    store_stream(act_e, act_ph, sem_dact)

            @block.vector
            def _(v):
                v.memset(xs[:], 0.0)
                for ph in range(8):
                    # diag j = 8t + ph of item (p, n) -> xs[p, ph, n, t, ph]
                    i = v.tensor_copy(
                        xs[:, ph, :, :, ph],
                        xall[:, :, ph : ph + 8 * (T - 1) + 1 : 8],
                    )
                    if ph == 0:
                        i.wait_op(sem_x, 16, "sem-ge")
                    i.then_inc(sem_xs, 1)

            @block.sync
            def _(sp_e):
                store_stream(sp_e, sp_ph, sem_dsp)
                if tiny is not None:
                    dt_ = sp_e.dma_start(out=tiny[:], in_=xall[:, 0, 0:1])
                    dt_.then_inc(sem_t, 16)
                    sp_e.wait_ge(sem_t, 16)

    return nc


def _get_amx(repeat: int = 1, timing: bool = False, single_packet: bool = False,
             sp_ph=(0, 1, 2, 3), act_ph=(4, 5, 6, 7)):
    key = ("amx", repeat, timing, single_packet, tuple(sp_ph), tuple(act_ph))
    if key not in _prog_cache:
        _prog_cache[key] = _build_amx(repeat, timing, single_packet, sp_ph, act_ph)
    return _prog_cache[key]


def _build_amxb(repeat: int = 1, timing: bool = False,
                 single_packet: bool = False):
    """amx + item-boundary descriptor merge.

    diag 175 of item b (last float of its flat [176*176] row) and
    diag 0 of item b+1 (first float of the next row) are ADJACENT
    floats in DRAM.  For the 7 within-partition item boundaries
    (items 8p+n-1 -> 8p+n, n=1..7) both values ride ONE 64 B
    descriptor (two full 32 B beats, no RMW) instead of two 32 B
    ones: 896 of 180224 descriptors saved per core per iteration.

    Store DMAs per iteration (vs amx's 8):
      ph 1..6   : unchanged [1024, 22, 8]           6 x 22528 descs
      ph 0 main : t=1..21 all items [1024, 21, 8]       21504
      ph 7 main : t=0..20 all items [1024, 21, 8]       21504
      rem0      : diag 0 of items 8p   [128, 1, 8]        128
      rem7      : diag 175 of items 8p+7 [128, 1, 8]      128
      bnd n=1..7: [128, 16] straddling the item seam   7 x 128
    Total 179328 descriptors in 17 DMAs.
    """
    from concourse import bass, mybir

    f32 = mybir.dt.float32
    nc = bass.Bass(target_bir_lowering=False)
    NS = N_CHUNKS                  # 8 item slots per partition
    T = D // 8                     # 22 windows per phase per item

    x = nc.dram_tensor("x", [B_SHARD, D], f32, kind="ExternalInput")
    if timing:
        out = nc.dram_tensor("outscratch", [B_SHARD, D, D], f32)
        tiny = nc.dram_tensor("tiny_out", [P, 1], f32, kind="ExternalOutput")
    else:
        out = nc.dram_tensor("out", [B_SHARD, D, D], f32, kind="ExternalOutput")
        tiny = None
    outw = out[:].rearrange("b i j -> b (i j)").rearrange(
        "b (t f) -> b t f", f=8
    )                                             # [1024, 3872, 8]
    # per-partition flat float view: partition p owns items [8p, 8p+8)
    outp = out[:].rearrange("b i j -> (b i j)").rearrange(
        "(p r) -> p r", p=P
    )                                             # [128, 247808]

    import contextlib

    with contextlib.ExitStack() as ctx:
        sem_x = ctx.enter_context(nc.semaphore("sem_x"))
        sem_t = ctx.enter_context(nc.semaphore("sem_t"))
        sem_xs = ctx.enter_context(nc.semaphore("sem_xs"))
        sem_dsp = ctx.enter_context(nc.semaphore("sem_dsp"))
        sem_dact = ctx.enter_context(nc.semaphore("sem_dact"))
        xall = ctx.enter_context(nc.sbuf_tensor("xall", [P, NS, D], f32))
        xs = ctx.enter_context(nc.sbuf_tensor("xs", [P, 8, NS, T, 8], f32))
        # phase-0 t=1..21 / phase-7 t=0..20 bands, packed (n, t, f)
        xs0 = ctx.enter_context(nc.sbuf_tensor("xs0", [P, NS, T - 1, 8], f32))
        xs7 = ctx.enter_context(nc.sbuf_tensor("xs7", [P, NS, T - 1, 8], f32))
        xr0 = ctx.enter_context(nc.sbuf_tensor("xr0", [P, 8], f32))
        xr7 = ctx.enter_context(nc.sbuf_tensor("xr7", [P, 8], f32))
        xb = ctx.enter_context(nc.sbuf_tensor("xb", [P, NS - 1, 16], f32))
        N_STAGE = 12   # sem_xs incs: 6 xs + xs0 + xs7 + xr0 + xr7 + 2 xb

        with nc.Block(no_gpsimd_drain=True) as block:

            def dmas_sp(eng):
                # phase-0 main band: windows at group 177t, t=1..21
                yield eng.dma_start(
                    out=outw[:, 177 : 177 * (T - 1) + 1 : 177, :], in_=xs0[:],
                    single_packet=single_packet,
                )
                for ph in (1, 2, 3):
                    yield eng.dma_start(
                        out=outw[:, 22 * ph : 177 * (T - 1) + 22 * ph + 1 : 177, :],
                        in_=xs[:, ph], single_packet=single_packet,
                    )
                # diag 0 of items 8p
                yield eng.dma_start(out=outw[0:B_SHARD:NS, 0:1, :], in_=xr0[:],
                                    single_packet=single_packet)
                for n in (1, 2, 3):
                    yield eng.dma_start(
                        out=outp[:, DD * n - 8 : DD * n + 8], in_=xb[:, n - 1],
                        single_packet=single_packet,
                    )

            def dmas_act(eng):
                for ph in (4, 5, 6):
                    yield eng.dma_start(
                        out=outw[:, 22 * ph : 177 * (T - 1) + 22 * ph + 1 : 177, :],
                        in_=xs[:, ph], single_packet=single_packet,
                    )
                # phase-7 main band: windows at group 177t + 154, t=0..20
                yield eng.dma_start(
                    out=outw[:, 154 : 177 * (T - 2) + 155 : 177, :], in_=xs7[:],
                    single_packet=single_packet,
                )
                # diag 175 of items 8p+7
                yield eng.dma_start(
                    out=outw[NS - 1 : B_SHARD : NS, 3871:3872, :], in_=xr7[:],
                    single_packet=single_packet,
                )
                for n in (4, 5, 6, 7):
                    yield eng.dma_start(
                        out=outp[:, DD * n - 8 : DD * n + 8], in_=xb[:, n - 1],
                        single_packet=single_packet,
                    )

            def store_stream(eng, dmas_fn, sem_d):
                cnt = 0
                for r in range(repeat):
                    for k, dma in enumerate(dmas_fn(eng)):
                        if r == 0 and k == 0:
                            dma.wait_op(sem_xs, N_STAGE, "sem-ge")
                        dma.then_inc(sem_d, 16)
                        cnt += 1
                eng.wait_ge(sem_d, 16 * cnt)

            @block.scalar
            def _(act_e):
                act_e.dma_start(
                    out=xall[:],
                    in_=x[:].rearrange("(p n) d -> p (n d)", p=P),
                ).then_inc(sem_x, 16)
                store_stream(act_e, dmas_act, sem_dact)

            @block.vector
            def _(v):
                for buf in (xs, xs0, xs7, xr0, xr7, xb):
                    v.memset(buf[:], 0.0)
                first = True

                def stage(dst, src):
                    nonlocal first
                    i = v.tensor_copy(dst, src)
                    if first:
                        i.wait_op(sem_x, 16, "sem-ge")
                        first = False
                    i.then_inc(sem_xs, 1)

                for ph in range(1, 7):
                    # diag j = 8t + ph of item (p, n) -> xs[p, ph, n, t, ph]
                    stage(
                        xs[:, ph, :, :, ph],
                        xall[:, :, ph : ph + 8 * (T - 1) + 1 : 8],
                    )
                # phase-0 band: j = 8t, t=1..21 -> xs0[p, n, t-1, 0]
                stage(xs0[:, :, :, 0], xall[:, :, 8 : 8 * (T - 1) + 1 : 8])
                # phase-7 band: j = 8t+7, t=0..20 -> xs7[p, n, t, 7]
                stage(xs7[:, :, :, 7], xall[:, :, 7 : 7 + 8 * (T - 2) + 1 : 8])
                stage(xr0[:, 0:1], xall[:, 0, 0:1])
                stage(xr7[:, 7:8], xall[:, 7, D - 1 : D])
                # boundary seam n: x[8p+n-1, 175] @ 7 and x[8p+n, 0] @ 8
                stage(xb[:, :, 7:8], xall[:, 0 : NS - 1, D - 1 : D])
                stage(xb[:, :, 8:9], xall[:, 1:NS, 0:1])

            @block.sync
            def _(sp_e):
                store_stream(sp_e, dmas_sp, sem_dsp)
                if tiny is not None:
                    dt_ = sp_e.dma_start(out=tiny[:], in_=xall[:, 0, 0:1])
                    dt_.then_inc(sem_t, 16)
                    sp_e.wait_ge(sem_t, 16)

    return nc


def _get_amxb(repeat: int = 1, timing: bool = False, single_packet: bool = False):
    key = ("amxb", repeat, timing, single_packet)
    if key not in _prog_cache:
        _prog_cache[key] = _build_amxb(repeat, timing, single_packet)
    return _prog_cache[key]


# Primary on-device program: set from slope-bench results.  The winner
# is the fully-aligned diag scatter with the transposed item->partition
# map ("amx"): every diag value is one 32 B-aligned full-word DMA
# descriptor into pre-zeroed output, the whole 1024-item shard is 8
# store DMAs (4 per HWDGE ring).  HW slope (r2=801, 24 pairs):
#   amx 109.2 us ~= ams 109.2 < hy176al 112.5 << sc4B 427.7 (HBM RMW)
#   << dense 353 us (HBM-bw bound).
# All 32 B-scatter layouts sit on the same per-descriptor SDMA floor
# (~9.7 ns/desc/engine * 180224 descs / 16 engines); instruction-count
# reduction below ~64 DMAs buys nothing more, but amx compiles far
# smaller timing NEFFs.
# amxb (= amx + item-seam 64 B merge, -896 descs) ties amx within
# noise once single_packet is on (32-pair medians 109.52 vs 109.25):
# the 64 B seam writes issue as two 32 B beats, refunding the saving.
# Ship the simpler amx with single_packet=True.
# ("amx",) | ("amxb",) | ("ams",) | ("hybrid", ...) | ("dense",)
PRIMARY = ("amx",)


def _get_primary_program(repeat: int = 1, timing: bool = False):
    if PRIMARY[0] == "hybrid":
        _, a, sd, al = PRIMARY
        return _get_hybrid_program(repeat, timing, a, sd, al)
    if PRIMARY[0] == "ams":
        return _get_am_split(repeat, timing)
    if PRIMARY[0] == "amx":
        # single_packet packs each DMA's descriptor stream into larger
        # ring packets (~0.5%); all stores on the single SP ring avoids
        # cross-ring packet switches (108.66 vs 108.93 paired medians)
        return _get_amx(repeat, timing, single_packet=True,
                        sp_ph=tuple(range(8)), act_ph=())
    if PRIMARY[0] == "amxb":
        return _get_amxb(repeat, timing, single_packet=True)
    return _get_program(repeat, timing)


def _exec(nc, x: np.ndarray, **spmd_kwargs):
    from concourse.bass_utils import run_bass_kernel_spmd

    in_maps = [
        {"x": x[c * B_SHARD : (c + 1) * B_SHARD]} for c in range(N_CORES)
    ]
    res = run_bass_kernel_spmd(nc, in_maps, list(range(N_CORES)), **spmd_kwargs)
    full = np.concatenate([r["out"] for r in res.results], axis=0)
    return full, res


def _run(x: np.ndarray, **spmd_kwargs):
    x = np.ascontiguousarray(x, dtype=np.float32)
    assert x.shape == (B_FULL, D), x.shape
    full, res = _exec(_get_primary_program(), x, **spmd_kwargs)
    if PRIMARY[0] != "dense":
        # The scatter/hybrid programs rely on the runtime zero-filling
        # ExternalOutput DRAM (bass2jax binds np.zeros to the out tensor;
        # native run_bass_kernel_spmd pre-zeros out buffers).  Verify
        # that contract held — diag must equal x exactly and everything
        # off-diag must be zero — and fall back to the fully dense
        # writer if not.
        idx = np.arange(D)
        diag = full[:, idx, idx]
        if not (
            (diag == x).all()
            and np.count_nonzero(full) == np.count_nonzero(x)
        ):
            full, res = _exec(_get_program(), x, **spmd_kwargs)
    return full, res


def kernel(**inputs) -> np.ndarray:
    full, _ = _run(inputs["x"])
    return full

